# revision 11
# baseline (speedup 1.0000x reference)
"""Bass/Trainium2 kernel v2 for nn_LSTMModel (B=128, T=512, D=256, H=512).

Data-parallel over batch across 8 cores (16 rows each), weights replicated
SBUF-resident bf16 transposed (lhsT), g-gate rows pre-scaled 2x so
tanh(g) = 2*sigmoid(2g) - 1 needs only the sigmoid table.

v2 structure (vs v1):
- Gate PSUM groups are STARTED by a PE identity-matmul "inject" of the
  precomputed x-side block (xg), so no DVE add sits on the chain.
- L1 runs TWO 16-step blocks behind L0. Its x-side (wx1 @ h0) is computed
  per block with wide N=256 matmuls from an h0 block buffer (h0blk) that
  the L0 chain writes directly (strided dst), then injected per step like
  xg0. Only the two h-side recurrences use small N=16 matmuls.
- xg copies PSUM->SBUF ride ACT (xg0, bias fused via activation bias) and
  DVE (xg1, bias fused via tensor_scalar_add), one mc per step.
- Single PSUM bank per layer-step; one sigmoid ACT covers i|f|2g|o.
"""

import numpy as np

import concourse.bass as bass
import concourse.tile as tile
import concourse.mybir as mybir
from concourse import bacc
from concourse.bass import ds
from concourse.bass_utils import run_bass_kernel_spmd
from concourse.masks import make_identity

F32 = mybir.dt.float32
F32R = mybir.dt.float32r
BF16 = mybir.dt.bfloat16
FP8 = mybir.dt.float8e4
DR = mybir.MatmulPerfMode.DoubleRow
AF = mybir.ActivationFunctionType
OP = mybir.AluOpType

B, T, D, H = 128, 512, 256, 512
NCORES = 8
BL = B // NCORES            # 16
G = 4 * H                   # 2048
MCH = G // 128              # 16
DKC = D // 128              # 2
HKC = H // 128              # 4
U = 16                      # steps per block
PRIO_L0 = 0                 # priority offset for the L0 chain
SPLIT_SIG0 = False          # split sigma0 into two 128-col halves
TMP_POOL = False            # gpsimd can't run TensorScalarPtr on HW (walrus check)
TMP1_POOL = False           # same restriction
TG1_POOL = True             # L1 cprod (tensor_mul, HW-legal) on gpsimd
CHAIN_ORDER = "l0_first"    # emission order of the two chains per step
H1_POOL = False             # h1 on gpsimd lengthens the L1 cycle
H0_POOL = False             # h0 write on gpsimd: pool launch hurts the edge
FP8_H = False               # fp8e4 DoubleRow h-streams: fast but too lossy
PROD_PLACE = 1              # bit1: xg1 prod between streams; bit2: xg0
XG0_COPY = "act"            # act | dve: engine for the xg0 PSUM->SBUF copy
XG_DMA = False              # (dead: DMA cannot read PSUM)
                            # injects read f32r at 1 cyc/row
UB = U * BL                 # xg cols per block (u, b)
# source gate chunk gc (i:0-3 f:4-7 g:8-11 o:12-15) -> dest mc chunk,
# giving PSUM gate layout [g|i|f|o] so sigma can split [g|i] then [f|o]
PERM = {8: 0, 9: 1, 10: 2, 11: 3, 0: 4, 1: 5, 2: 6, 3: 7,
        4: 8, 5: 9, 6: 10, 7: 11, 12: 12, 13: 13, 14: 14, 15: 15}


def build_nc(t_steps=T, mode="real", staggered=True):
    assert t_steps % (2 * U) == 0
    nblk = t_steps // U
    n_iter = nblk // 2
    ntot = t_steps * BL
    nch = 512 if ntot % 512 == 0 else ntot

    nc = bacc.Bacc("TRN2", target_bir_lowering=False)

    x = nc.dram_tensor("x", [BL, t_steps, D], F32, kind="ExternalInput")
    proj_w = nc.dram_tensor("proj_w", [D, D], F32, kind="ExternalInput")
    proj_b = nc.dram_tensor("proj_b", [D], F32, kind="ExternalInput")
    wx0 = nc.dram_tensor("wx0", [G, D], F32, kind="ExternalInput")
    bx0 = nc.dram_tensor("bx0", [G], F32, kind="ExternalInput")
    wh0 = nc.dram_tensor("wh0", [G, H], F32, kind="ExternalInput")
    bh0 = nc.dram_tensor("bh0", [G], F32, kind="ExternalInput")
    wx1 = nc.dram_tensor("wx1", [G, H], F32, kind="ExternalInput")
    bx1 = nc.dram_tensor("bx1", [G], F32, kind="ExternalInput")
    wh1 = nc.dram_tensor("wh1", [G, H], F32, kind="ExternalInput")
    bh1 = nc.dram_tensor("bh1", [G], F32, kind="ExternalInput")
    fc1_w = nc.dram_tensor("fc1_w", [32, H], F32, kind="ExternalInput")
    fc1_b = nc.dram_tensor("fc1_b", [32], F32, kind="ExternalInput")
    fc2_w = nc.dram_tensor("fc2_w", [1, 32], F32, kind="ExternalInput")
    fc2_b = nc.dram_tensor("fc2_b", [1], F32, kind="ExternalInput")
    out_d = nc.dram_tensor("out", [BL, 1], F32, kind="ExternalOutput")

    tens = dict(locals())
    with tile.TileContext(nc) as tc:
        with tc.tile_pool(name="res", bufs=1) as res, \
             tc.tile_pool(name="stg", bufs=3) as stg, \
             tc.tile_pool(name="scn", bufs=3) as scn, \
             tc.tile_pool(name="xsp", bufs=2) as xsp, \
             tc.tile_pool(name="psum", bufs=2, space="PSUM") as psum, \
             tc.tile_pool(name="psx", bufs=3, space="PSUM") as psx:
            _build_body(nc, tc, res, stg, scn, xsp, psum, psx, tens, t_steps,
                        nblk, n_iter, ntot, nch, mode, staggered)
    nc.compile()
    return nc


def _build_body(nc, tc, res, stg, scn, xsp, psum, psx, tens, t_steps, nblk,
                n_iter, ntot, nch, mode, staggered):
    x, out_d = tens["x"], tens["out_d"]

    def act(fn, dst, src_, **kw):
        nc.scalar.activation(dst, src_, fn, **kw)

    ident = res.tile([128, 128], F32, tag="ident")
    make_identity(nc, ident[:, :])
    identb = res.tile([128, 128], BF16, tag="identb")
    nc.vector.tensor_copy(identb[:, :], ident[:, :])

    def big():
        return psx.tile([128, 512], F32, tag="big", name="big")

    # ---- resident transposed weights (bf16); g-gate rows pre-scaled 2x ----
    HDT = FP8 if FP8_H else BF16
    wx0T = res.tile([128, DKC * G], BF16, tag="wx0T")
    wh0T = res.tile([128, HKC * G], HDT, tag="wh0T")
    wx1T = res.tile([128, HKC * G], BF16, tag="wx1T")
    wh1T = res.tile([128, HKC * G], HDT, tag="wh1T")
    def load_w(w_d, kcs, dst, rec):
        cdim = w_d.shape[1]
        for gc0 in range(0, MCH, 2):
            st = stg.tile([128, 2 * 512], F32, tag="wstage")
            stv = st[:, :].rearrange("p (g c) -> p g c", g=2)
            nc.sync.dma_start(
                out=stv[:, :, 0:cdim],
                in_=w_d[gc0 * 128:(gc0 + 2) * 128, :].rearrange(
                    "(g p) c -> p g c", p=128))
            for gi in range(2):
                gc = gc0 + gi
                for kc in range(kcs):
                    pt = big()
                    nc.tensor.transpose(
                        pt[:, 0:128],
                        st[:, gi * 512 + kc * 128:gi * 512 + kc * 128 + 128],
                        ident[:, :])
                    if rec and FP8_H:
                        o = (((kc // 2) * MCH + PERM[gc]) * 2 + kc % 2) * 128
                    else:
                        o = (kc * MCH + PERM[gc]) * 128
                    if 8 <= gc <= 11:   # tanh(x) = 2*sigmoid(2x) - 1
                        nc.vector.tensor_scalar_mul(dst[:, o:o + 128],
                                                    pt[:, 0:128], 2.0)
                    elif (gc + kc) % 2 == 0:
                        act(AF.Copy, dst[:, o:o + 128], pt[:, 0:128])
                    else:
                        nc.vector.tensor_copy(dst[:, o:o + 128],
                                              pt[:, 0:128])

    projT = res.tile([128, 2 * D], BF16, tag="projT")
    for gc in range(DKC):
        st = stg.tile([128, 512], F32, tag="wstage")
        nc.sync.dma_start(out=st[:, 0:D],
                          in_=tens["proj_w"][gc * 128:(gc + 1) * 128, :])
        for kc in range(DKC):
            pt = big()
            nc.tensor.transpose(pt[:, 0:128],
                                st[:, kc * 128:(kc + 1) * 128], ident[:, :])
            nc.vector.tensor_copy(projT[:, (kc * 2 + gc) * 128:
                                        (kc * 2 + gc) * 128 + 128],
                                  pt[:, 0:128])

    fc1T = res.tile([128, HKC * 32], BF16, tag="fc1T")
    st = stg.tile([128, 512], F32, tag="wstage")
    nc.sync.dma_start(out=st[0:32, :], in_=tens["fc1_w"][:, :])
    for kc in range(HKC):
        pt = big()
        nc.tensor.transpose(pt[:, 0:32], st[0:32, kc * 128:(kc + 1) * 128],
                            ident[0:32, 0:32])
        nc.vector.tensor_copy(fc1T[:, kc * 32:(kc + 1) * 32], pt[:, 0:32])
    fc2T_f = res.tile([32, 1], F32, tag="fc2T_f")
    nc.sync.dma_start(out=fc2T_f[:, :],
                      in_=tens["fc2_w"][0:1, :].rearrange("o k -> k o"))
    fc2T = res.tile([32, 1], BF16, tag="fc2T")
    nc.vector.tensor_copy(fc2T[:, :], fc2T_f[:, :])
    fc1b = res.tile([32, 1], F32, tag="fc1b")
    nc.sync.dma_start(out=fc1b[:, :],
                      in_=tens["fc1_b"][:].rearrange("(k o) -> k o", o=1))
    fc2b = res.tile([1, 1], F32, tag="fc2b")
    nc.sync.dma_start(out=fc2b[:, :],
                      in_=tens["fc2_b"][:].rearrange("(k o) -> k o", o=1))

    # ---- gate biases: bsum[p, m] = (bx+bh)[m*128+p]; g region scaled 2x ----
    bsums = []
    for ba, bb in ((tens["bx0"], tens["bh0"]), (tens["bx1"], tens["bh1"])):
        parts = []
        for src in (ba, bb):
            st = stg.tile([16, 128], F32, tag="bstage")
            nc.sync.dma_start(out=st[:, :],
                              in_=src[:].rearrange("(m p) -> m p", p=128))
            pt = big()
            nc.tensor.transpose(pt[:, 0:16], st[:, :], ident[0:16, 0:16])
            sb = stg.tile([128, 16], F32, tag="btp")
            nc.vector.tensor_copy(sb[:, :], pt[:, 0:16])
            parts.append(sb)
        tot = stg.tile([128, 16], F32, tag="bsumraw")
        nc.vector.tensor_add(tot[:, :], parts[0][:, :], parts[1][:, :])
        nc.vector.tensor_scalar_mul(tot[:, 8:12], tot[:, 8:12], 2.0)
        totp = res.tile([128, 16], F32, tag=f"bsum{len(bsums)}")
        nc.vector.tensor_copy(totp[:, 0:4], tot[:, 8:12])
        nc.vector.tensor_copy(totp[:, 4:8], tot[:, 0:4])
        nc.vector.tensor_copy(totp[:, 8:12], tot[:, 4:8])
        nc.vector.tensor_copy(totp[:, 12:16], tot[:, 12:16])
        bsums.append(totp)

    # ---- x -> xT (bf16), column order n = t*16 + b ----
    xT = res.tile([128, DKC * ntot], BF16, tag="xT")
    # one DMA per 8 steps onto 16 partitions (plain partition dim; split
    # partition dst DMAs scramble on HW), then narrow PE transposes
    for rc in range(t_steps // 8):
        stxB = xsp.tile([16, 8 * 256], F32, tag="xstage")
        nc.sync.dma_start(
            out=stxB[:, :].rearrange("b (t d) -> b t d", t=8),
            in_=x[:, rc * 8:(rc + 1) * 8, :])
        for kc in range(DKC):
            pt = big()
            for tl in range(8):
                o = tl * 256 + kc * 128
                nc.tensor.transpose(pt[:, tl * 16:(tl + 1) * 16],
                                    stxB[0:16, o:o + 128],
                                    ident[0:16, 0:16])
            dst = xT[:, kc * ntot + rc * 128:kc * ntot + rc * 128 + 128]
            if (rc + kc) % 2 == 0:
                act(AF.Copy, dst, pt[:, 0:128])
            else:
                nc.vector.tensor_copy(dst, pt[:, 0:128])

    stp = stg.tile([2, 128], F32, tag="bstage2")
    nc.sync.dma_start(out=stp[0:2, :],
                      in_=tens["proj_b"][:].rearrange("(m p) -> m p", p=128))
    ptp = big()
    nc.tensor.transpose(ptp[:, 0:2], stp[0:2, :], ident[0:2, 0:2])
    projb_t = res.tile([128, 2], F32, tag="projb")
    nc.vector.tensor_copy(projb_t[:, :], ptp[:, 0:2])

    # ---- xp = x @ proj_w.T + proj_b -> bf16 resident, one block of pad ----
    ntot2 = ntot + UB
    xp = res.tile([128, DKC * ntot2], BF16, tag="xp")
    for kc in range(DKC):
        nc.vector.memset(xp[:, kc * ntot2 + ntot:(kc + 1) * ntot2], 0.0)
    for nt in range(ntot // nch):
        for mc in range(DKC):
            px = big()
            for kc in range(DKC):
                nc.tensor.matmul(
                    px[:, 0:nch],
                    projT[:, (kc * 2 + mc) * 128:(kc * 2 + mc) * 128 + 128],
                    xT[:, kc * ntot + nt * nch:kc * ntot + (nt + 1) * nch],
                    start=(kc == 0), stop=(kc == DKC - 1))
            if (nt + mc) % 2 == 0:
                act(AF.Identity,
                    xp[:, mc * ntot2 + nt * nch:mc * ntot2 + (nt + 1) * nch],
                    px[:, 0:nch], bias=projb_t[:, mc:mc + 1])
            else:
                nc.vector.tensor_scalar_add(
                    xp[:, mc * ntot2 + nt * nch:mc * ntot2 + (nt + 1) * nch],
                    px[:, 0:nch], projb_t[:, mc:mc + 1])

    # weight loads emitted after the x pipeline so the SP/DMA queue feeds
    # the long pole (x -> xT -> xp -> xg0) first; these overlap it
    load_w(tens["wx0"], DKC, wx0T, False)
    load_w(tens["wh0"], HKC, wh0T, True)

    # ---- scan state ----
    c0 = res.tile([128, 64], F32, tag="c0")
    c1 = res.tile([128, 64], F32, tag="c1")
    h1 = res.tile([128, 64], HDT, tag="h1")
    # h0 lives in per-block buffers, col = j*UB + u*16 + b
    h0R = res.tile([128, HKC * UB], HDT, tag="h0R")
    h0S = res.tile([128, HKC * UB], HDT, tag="h0S")
    XGDT = F32 if XG_DMA else BF16
    xg0A = res.tile([128, U * 256], XGDT, tag="xg0A")
    xg0B = res.tile([128, U * 256], XGDT, tag="xg0B")
    xg1P = res.tile([128, U * 256], XGDT, tag="xg1P")
    xg1Q = res.tile([128, U * 256], XGDT, tag="xg1Q")
    if XG_DMA:
        # bsumT[k, m] = bsum[m, k]; indic[k, n] = (n // 16 == k)
        bsumTs = []
        for i in range(2):
            ptb = big()
            nc.tensor.transpose(ptb[0:16, 0:128], bsums[i][:, :],
                                ident[:, :])
            bt = res.tile([16, 128], F32, tag=f"bsumT{i}")
            nc.vector.tensor_copy(bt[:, :], ptb[0:16, 0:128])
            bsumTs.append(bt)
        indic = res.tile([16, 256], F32, tag="indic")
        nc.vector.memset(indic[:, :], 0.0)
        for k in range(16):
            nc.vector.memset(indic[k:k + 1, k * 16:(k + 1) * 16], 1.0)
    for s_ in (c0, c1, h1, h0R, h0S):
        nc.vector.memset(s_[:, :], 0.0)

    def j3(ap):
        return ap.rearrange("p (j b) -> p j b", j=HKC)

    def stage_xpb(off):
        # one block of xp columns -> static staging tile (dynamic offset)
        xpb = scn.tile([128, DKC * UB], BF16, tag="xpb", name="xpb")
        srcv = xp[:, :].rearrange("p (k n) -> p k n", k=DKC)
        sel = srcv[:, :, off:off + UB] if isinstance(off, int) \
            else srcv[:, :, ds(off, UB)]
        nc.sync.dma_start(
            out=xpb[:, :].rearrange("p (k n) -> p k n", k=DKC),
            in_=sel)
        return xpb

    def emit_xg0_piece(xpb, dst_xg, mc):
        # dst_xg[:, u*256 + mc*16 + b] = (xp @ wx0.T)[mc chunk] + bias (ACT)
        pt = psx.tile([128, 512], F32, tag="big", name="xg0p")
        for kc in range(DKC):
            nc.tensor.matmul(
                pt[:, 0:UB],
                wx0T[:, (kc * MCH + mc) * 128:(kc * MCH + mc) * 128 + 128],
                xpb[:, kc * UB:(kc + 1) * UB],
                start=(kc == 0), stop=(kc == DKC - 1))
        xgv = dst_xg[:, :].rearrange("p (u r) -> p u r", r=256)
        if XG_DMA:
            nc.sync.dma_start(
                out=xgv[:, :, mc * 16:(mc + 1) * 16],
                in_=pt[:, 0:UB].rearrange("p (u b) -> p u b", b=16))
        elif XG0_COPY == "act":
            act(AF.Identity, xgv[:, :, mc * 16:(mc + 1) * 16],
                pt[:, 0:UB].rearrange("p (u b) -> p u b", b=16),
                bias=bsums[0][:, mc:mc + 1])
        else:
            nc.vector.tensor_scalar_add(
                xgv[:, :, mc * 16:(mc + 1) * 16],
                pt[:, 0:UB].rearrange("p (u b) -> p u b", b=16),
                bsums[0][:, mc:mc + 1])

    def emit_xg1_piece(src_blk, dst_xg, mc):
        # dst_xg[:, u*256 + mc*16 + b] = (h0blk @ wx1.T)[mc chunk] + bias (DVE)
        pt = psx.tile([128, 512], F32, tag="big", name="xg1p")
        for j in range(HKC):
            nc.tensor.matmul(
                pt[:, 0:UB],
                wx1T[:, (j * MCH + mc) * 128:(j * MCH + mc) * 128 + 128],
                src_blk[:, j * UB:(j + 1) * UB],
                start=(j == 0), stop=(j == HKC - 1))
        xgv = dst_xg[:, :].rearrange("p (u r) -> p u r", r=256)
        if XG_DMA:
            nc.sync.dma_start(
                out=xgv[:, :, mc * 16:(mc + 1) * 16],
                in_=pt[:, 0:UB].rearrange("p (u b) -> p u b", b=16))
        else:
            nc.vector.tensor_scalar_add(
                xgv[:, :, mc * 16:(mc + 1) * 16],
                pt[:, 0:UB].rearrange("p (u b) -> p u b", b=16),
                bsums[1][:, mc:mc + 1])

    def emit_l0_mms(ps0, cur_xg0, cur_blk, prv_blk, u):
        if XG_DMA:
            nc.tensor.matmul(ps0[:, 0:256], ident[:, :].bitcast(F32R),
                             cur_xg0[:, u * 256:(u + 1) * 256].bitcast(F32R),
                             start=True, stop=False)
            nc.tensor.matmul(ps0[:, 0:256], bsumTs[0][:, :].bitcast(F32R),
                             indic[:, :].bitcast(F32R),
                             start=False, stop=False)
        else:
            nc.tensor.matmul(ps0[:, 0:256], identb[:, :],
                             cur_xg0[:, u * 256:(u + 1) * 256],
                             start=True, stop=False)
        src_blk, tu = (prv_blk, U - 1) if u == 0 else (cur_blk, u - 1)
        # j-outer: the first 16 matmuls only need h0's j=0 quarter, which
        # the split hmul in chain_l0 produces first.
        if FP8_H:
            sv = src_blk[:, :].rearrange("p (j n) -> p j n", j=HKC)
            for jp in range(HKC // 2):
                for mc in range(MCH):
                    o = ((jp * MCH + mc) * 2) * 128
                    nc.tensor.matmul(
                        ps0[:, mc * 16:(mc + 1) * 16],
                        wh0T[:, o:o + 256].rearrange("p (t n) -> p t n", t=2),
                        sv[:, 2 * jp:2 * jp + 2, tu * 16:(tu + 1) * 16],
                        start=False,
                        stop=(jp == HKC // 2 - 1 and mc == MCH - 1),
                        perf_mode=DR)
        else:
            for j in range(HKC):
                for mc in range(MCH):
                    nc.tensor.matmul(
                        ps0[:, mc * 16:(mc + 1) * 16],
                        wh0T[:, (j * MCH + mc) * 128:
                             (j * MCH + mc) * 128 + 128],
                        src_blk[:, j * UB + tu * 16:j * UB + tu * 16 + 16],
                        start=False,
                        stop=(j == HKC - 1 and mc == MCH - 1))

    def emit_l1_mms(ps1, cur_xg1, u, tail=False):
        if XG_DMA:
            nc.tensor.matmul(ps1[:, 0:256], ident[:, :].bitcast(F32R),
                             cur_xg1[:, u * 256:(u + 1) * 256].bitcast(F32R),
                             start=True, stop=False)
            nc.tensor.matmul(ps1[:, 0:256], bsumTs[1][:, :].bitcast(F32R),
                             indic[:, :].bitcast(F32R),
                             start=False, stop=False)
        else:
            nc.tensor.matmul(ps1[:, 0:256], identb[:, :],
                             cur_xg1[:, u * 256:(u + 1) * 256],
                             start=True, stop=False)
        if FP8_H:
            hv = h1[:, :].rearrange("p (j n) -> p j n", j=HKC)
            for jp in range(HKC // 2):
                for mc in range(MCH):
                    o = ((jp * MCH + mc) * 2) * 128
                    nc.tensor.matmul(
                        ps1[:, mc * 16:(mc + 1) * 16],
                        wh1T[:, o:o + 256].rearrange("p (t n) -> p t n", t=2),
                        hv[:, 2 * jp:2 * jp + 2, :],
                        start=False,
                        stop=(jp == HKC // 2 - 1 and mc == MCH - 1),
                        perf_mode=DR)
        elif tail:
            for j in range(HKC):
                for mc in range(MCH):
                    nc.tensor.matmul(
                        ps1[:, mc * 16:(mc + 1) * 16],
                        wh1T[:, (j * MCH + mc) * 128:
                             (j * MCH + mc) * 128 + 128],
                        h1[:, j * 16:(j + 1) * 16],
                        start=False,
                        stop=(j == HKC - 1 and mc == MCH - 1))
        else:
            for mc in range(MCH):
                for j in range(HKC):
                    nc.tensor.matmul(
                        ps1[:, mc * 16:(mc + 1) * 16],
                        wh1T[:, (j * MCH + mc) * 128:
                             (j * MCH + mc) * 128 + 128],
                        h1[:, j * 16:(j + 1) * 16],
                        start=False,
                        stop=(j == HKC - 1 and mc == MCH - 1))

    def chain_l0(ps0, cur_blk, u):
        # gates [2g|i|f|o]; split sigma: [g|i] first so the DVE chain
        # starts ~150ns earlier; [f|o] lands before cmul needs it.
        sig = scn.tile([128, 256], F32, tag="sig0")
        tg = scn.tile([128, 64], F32, tag="tg0")
        tc_ = scn.tile([128, 64], F32, tag="tc0")
        tmp = scn.tile([128, 64], F32, tag="tmp0")
        if SPLIT_SIG0 == "gif":
            act(AF.Sigmoid, sig[:, 0:192], ps0[:, 0:192])
            act(AF.Sigmoid, sig[:, 192:256], ps0[:, 192:256])
        elif SPLIT_SIG0:
            act(AF.Sigmoid, sig[:, 0:128], ps0[:, 0:128])
            act(AF.Sigmoid, sig[:, 128:256], ps0[:, 128:256])
        else:
            act(AF.Sigmoid, sig[:, :], ps0[:, :])
        # tmp = (sig_g - 0.5)*sig_i = tanh(g)*i/2 ; c = 2*tmp + sig_f*c
        veng = nc.gpsimd if TMP_POOL else nc.vector
        veng.scalar_tensor_tensor(tmp[:, :], sig[:, 0:64], 0.5,
                                  sig[:, 64:128], OP.subtract, OP.mult)
        nc.vector.tensor_mul(tg[:, :], sig[:, 128:192], c0[:, :])
        nc.vector.scalar_tensor_tensor(c0[:, :], tmp[:, :], 2.0,
                                       tg[:, :], OP.mult, OP.add)
        act(AF.Tanh, tc_[:, :], c0[:, :])
        dstv = cur_blk[:, :].rearrange("p (j n) -> p j n", j=HKC)
        heng = nc.gpsimd if H0_POOL else nc.vector
        # split: j=0 quarter first so the next step's j=0 matmuls can start
        heng.tensor_mul(dstv[:, 0:1, u * 16:(u + 1) * 16],
                        j3(sig[:, 192:256])[:, 0:1, :],
                        j3(tc_[:, :])[:, 0:1, :])
        heng.tensor_mul(dstv[:, 1:HKC, u * 16:(u + 1) * 16],
                        j3(sig[:, 192:256])[:, 1:HKC, :],
                        j3(tc_[:, :])[:, 1:HKC, :])

    def chain_l1(ps1, tail=False):
        sig = scn.tile([128, 256], F32, tag="sig1")
        tg = scn.tile([128, 64], F32, tag="tg1")
        tc_ = scn.tile([128, 64], F32, tag="tc1")
        tmp = scn.tile([128, 64], F32, tag="tmp1")
        if tail:
            act(AF.Sigmoid, sig[:, 0:128], ps1[:, 0:128])
            act(AF.Sigmoid, sig[:, 128:256], ps1[:, 128:256])
        else:
            act(AF.Sigmoid, sig[:, :], ps1[:, :])
        nc.vector.scalar_tensor_tensor(tmp[:, :], sig[:, 0:64], 0.5,
                                       sig[:, 64:128], OP.subtract, OP.mult)
        (nc.gpsimd if TG1_POOL else nc.vector).tensor_mul(
            tg[:, :], sig[:, 128:192], c1[:, :])
        nc.vector.scalar_tensor_tensor(c1[:, :], tmp[:, :], 2.0,
                                       tg[:, :], OP.mult, OP.add)
        act(AF.Tanh, tc_[:, :], c1[:, :])
        if tail:
            nc.vector.tensor_mul(h1[:, 0:16], sig[:, 192:208], tc_[:, 0:16])
            nc.vector.tensor_mul(h1[:, 16:64], sig[:, 208:256],
                                 tc_[:, 16:64])
        else:
            (nc.gpsimd if H1_POOL else nc.vector).tensor_mul(
                h1[:, :], sig[:, 192:256], tc_[:, :])

    def emit_subblock(cur_xg0, nxt_xg0, xg0_off, cur_blk, prv_blk,
                      xg1_src, xg1_dst, cur_xg1, do_l1):
        xpb = stage_xpb(xg0_off)
        for u in range(U):
            ps0 = psum.tile([128, 256], F32, tag="ps0", name="ps0")
            emit_l0_mms(ps0, cur_xg0, cur_blk, prv_blk, u)
            if PROD_PLACE & 2:
                emit_xg0_piece(xpb, nxt_xg0, u)
            if PROD_PLACE & 1 and xg1_dst is not None:
                emit_xg1_piece(xg1_src, xg1_dst, u)
            if do_l1:
                ps1 = psum.tile([128, 256], F32, tag="ps1", name="ps1")
                emit_l1_mms(ps1, cur_xg1, u)
            # PE bubble fillers: next-block xg production, one mc per step
            if not PROD_PLACE & 2:
                emit_xg0_piece(xpb, nxt_xg0, u)
            if not PROD_PLACE & 1 and xg1_dst is not None:
                emit_xg1_piece(xg1_src, xg1_dst, u)
            if mode != "nochain":
                if CHAIN_ORDER == "l1_first" and do_l1:
                    chain_l1(ps1)
                with tc.high_priority(offset=PRIO_L0):
                    chain_l0(ps0, cur_blk, u)
                if CHAIN_ORDER != "l1_first" and do_l1:
                    chain_l1(ps1)

    # ---- prologue: xg0(0); peeled iteration 0 (no L1) ----
    xpb0 = stage_xpb(0)
    for mc in range(MCH):
        emit_xg0_piece(xpb0, xg0A, mc)
    emit_subblock(xg0A, xg0B, UB, h0R, h0S, None, None, None, False)
    # wx1T is first consumed by peel-b's xg1 production, wh1T by the main
    # loop's L1 streams; loading them here overlaps the peeled blocks' scan
    load_w(tens["wx1"], HKC, wx1T, False)
    emit_subblock(xg0B, xg0A, 2 * UB, h0S, h0R, h0R, xg1P, None, False)
    load_w(tens["wh1"], HKC, wh1T, True)

    # ---- main loop: iteration k covers L0 blocks (2k, 2k+1),
    #      L1 blocks (2k-2, 2k-1); xg1(2k-1) from S, xg1(2k) from R ----
    if n_iter > 1:
        with tc.For_i(1, n_iter, 1,
                      hint_engines=(mybir.EngineType.PE,),
                      staggered_reset=staggered) as it:
            emit_subblock(xg0A, xg0B, it * (2 * UB) + UB,
                          h0R, h0S, h0S, xg1Q, xg1P, True)
            emit_subblock(xg0B, xg0A, it * (2 * UB) + 2 * UB,
                          h0S, h0R, h0R, xg1P, xg1Q, True)

    # ---- epilogue: xg1(last) from S; L1 blocks nblk-2 (P), nblk-1 (Q) ----
    for mc in range(MCH):
        emit_xg1_piece(h0S, xg1Q, mc)
    for cur_xg1 in (xg1P, xg1Q):
        for u in range(U):
            ps1 = psum.tile([128, 256], F32, tag="ps1", name="ps1")
            emit_l1_mms(ps1, cur_xg1, u)
            if mode != "nochain":
                chain_l1(ps1)

    # ---- FC head ----
    ph = big()
    for kc in range(HKC):
        nc.tensor.matmul(ph[0:32, 0:16], fc1T[:, kc * 32:(kc + 1) * 32],
                         h1[:, kc * 16:(kc + 1) * 16],
                         start=(kc == 0), stop=(kc == HKC - 1))
    hid = scn.tile([32, 16], BF16, tag="hid")
    nc.scalar.activation(hid[:, :], ph[0:32, 0:16], AF.Relu,
                         bias=fc1b[:, 0:1])
    po = big()
    nc.tensor.matmul(po[0:1, 0:16], fc2T[:, 0:1], hid[:, :],
                     start=True, stop=True)
    ob = scn.tile([1, 16], F32, tag="ob")
    nc.vector.tensor_scalar_add(ob[:, :], po[0:1, 0:16], fc2b[0:1, 0:1])
    nc.sync.dma_start(out=out_d[:, :].rearrange("b o -> o b"), in_=ob[:, :])


_NC_CACHE = {}


def _get_nc(t_steps=T):
    if t_steps not in _NC_CACHE:
        _NC_CACHE[t_steps] = build_nc(t_steps, "real", staggered=True)
    return _NC_CACHE[t_steps]


def kernel(**inputs):
    nc = _get_nc()
    arrs = {k: np.ascontiguousarray(np.asarray(v, dtype=np.float32))
            for k, v in inputs.items()}
    in_maps = []
    for c in range(NCORES):
        m = {k: v for k, v in arrs.items() if k != "x"}
        m["x"] = np.ascontiguousarray(arrs["x"][c * BL:(c + 1) * BL])
        in_maps.append(m)
    res = run_bass_kernel_spmd(nc, in_maps, core_ids=list(range(NCORES)))
    return np.concatenate([r["out"] for r in res.results], axis=0)
